# revision 55
# baseline (speedup 1.0000x reference)
"""Causal self-attention (B=2, T=2048, C=1024, H=16, RoPE) on 8 TRN2 cores.

Sharding: data-parallel over B (2 groups of 4 cores) x tensor-parallel over
heads (4 heads per core). Each core computes q/k/v projections for its heads,
RoPE, causal attention, and its partial output projection; the host sums the
4 partial projections per batch and adds bp.

Layout choices (per core):
  - x and weights arrive host-pre-tiled so every SBUF tile is a contiguous
    DRAM block (few, fat DMA descriptors); x lands in four 512-column
    blocks so projections start before the full load arrives.
  - q, k produced TRANSPOSED: qT/kT [256=4heads*64, T] via lhsT=W, rhs=xT.
    Head-dim pairs are pre-permuted (evens|odds) in the weights so RoPE
    needs no strided access; the pair-swap is a constant permutation
    matmul (J), combine on VectorE in bf16 (packed 2x mode).
  - v produced NON-transposed, per-head layout [v|1] (65 cols/head): the
    ones column rides along in each P@V matmul and produces the softmax
    denominator for free (psum row 64, no extra reduction).
  - scores computed transposed: ST[tk, tq] = k_rot @ q_rot^T per head; the
    two heads of a pair sit on partitions 0-63/64-127, so their score
    matmuls run CONCURRENTLY on the PE's 64x128 row tiles. Softmax-exp is
    elementwise (ScalarE, scale=1/8 folded in), the causal mask is a fixed
    128x128 triangle on diagonal blocks (GpSimd), fully-masked blocks are
    skipped, and diagonal blocks are column-trimmed in the score matmul,
    exp, and P@V (free dim starts at the diagonal).
  - softmax reciprocal on VectorE (reciprocal_approx_fast), broadcast over
    partitions with a constant matmul (EA).
  - the whole kernel is software-pipelined around ScalarE's exp stream
    (the steady-state pacer): scores+exp run LOOKAHEAD iterations ahead of
    their P@V consumers, each phase's softmax finalize is deferred past
    the next phase's score prologue, and projection work units (next x
    block's qkv proj, previous block's output proj) are interleaved into
    the attention kc loops to fill the PE's per-iteration slack.
"""

import math

import numpy as np
import ml_dtypes

import concourse.bass as bass
import concourse.bacc as bacc
import concourse.mybir as mybir
from concourse.tile import TileContext
from concourse.bass_utils import run_bass_kernel_spmd

BF16 = mybir.dt.bfloat16
F32 = mybir.dt.float32
NPBF16 = ml_dtypes.bfloat16

N_CORES = 8
P = 128

_UNIFIED_ACT_SET = "natural_log_exp_and_others"


def _patch_act_tables():
    import concourse.hw_specs as _hw
    import concourse.bacc as _bacc
    if getattr(_bacc, "_act_tables_patched", False):
        return
    _orig = _hw.get_activation_tables

    def _gat(arch):
        tabs = _orig(arch)
        if _UNIFIED_ACT_SET in tabs:
            keep = tabs[_UNIFIED_ACT_SET]
            drop = {
                mybir.ActivationFunctionType.Exp,
                mybir.ActivationFunctionType.Copy,
            } & keep
            for name, fns in tabs.items():
                if name != _UNIFIED_ACT_SET:
                    for f in drop:
                        fns.discard(f)
        return tabs

    _bacc.get_activation_tables = _gat
    _bacc._act_tables_patched = True


def build_attention_kernel(nc, T=2048, C=1024, n_heads=4, hd=64):
    """Emit the per-core kernel. Returns nothing; tensors are declared on nc."""
    _patch_act_tables()
    HD = n_heads * hd            # 256: local head dims
    KC = C // P                  # 8: contraction chunks for projections
    NJC = HD // P                # 2: partition tiles of qT/kT (head pairs)
    TQB = 512                    # tq block for scores/PV
    NQB = T // TQB               # 4
    NKC = T // P                 # 16: tk chunks
    VW = hd + 1                  # 65: v cols per head incl ones column
    PW = 2 * VW                  # 130: v cols per head pair
    scale = 1.0 / math.sqrt(hd)

    # ---- DRAM I/O ----
    # x and the weights arrive pre-tiled so every SBUF tile is one
    # contiguous DRAM block (big DMA descriptors, cheap triggers)
    xT = nc.declare_dram_parameter("xT", [NQB, P, KC * TQB], BF16,
                                   isOutput=False)
    wqT = nc.declare_dram_parameter("wqT", [P, KC * HD], BF16, isOutput=False)
    wkT = nc.declare_dram_parameter("wkT", [P, KC * HD], BF16, isOutput=False)
    wvT = nc.declare_dram_parameter("wvT", [P, KC * HD], BF16, isOutput=False)
    wpT = nc.declare_dram_parameter("wpT", [P, NJC * C], BF16, isOutput=False)
    # trig = [cos | sin], jte = [J | tri | EA], bqkv = [bq | bk | bv_row]
    trig = nc.declare_dram_parameter("trig", [P, 2 * T], BF16, isOutput=False)
    jte = nc.declare_dram_parameter("jte", [P, 3 * P], BF16, isOutput=False)
    bqkv = nc.declare_dram_parameter("bqkv", [P, 4 * NJC + HD], F32,
                                     isOutput=False)
    z = nc.declare_dram_parameter("z", [T, C], BF16, isOutput=True)

    with TileContext(nc) as tc:
        import contextlib

        with contextlib.ExitStack() as ctx:
            # ---- persistent SBUF pools ----
            pc = ctx.enter_context(tc.tile_pool(name="const", bufs=1))
            px = ctx.enter_context(tc.tile_pool(name="x", bufs=1))
            pw = ctx.enter_context(tc.tile_pool(name="w", bufs=1))
            pqk = ctx.enter_context(tc.tile_pool(name="qk", bufs=1))
            pv = ctx.enter_context(tc.tile_pool(name="v", bufs=1))
            py = ctx.enter_context(tc.tile_pool(name="y", bufs=1))
            # transient pools
            praw = ctx.enter_context(tc.tile_pool(name="raw", bufs=2))
            prt = ctx.enter_context(tc.tile_pool(name="ropetmp", bufs=4))
            pexp = ctx.enter_context(tc.tile_pool(name="exp", bufs=6))
            prcp = ctx.enter_context(tc.tile_pool(name="rcp", bufs=3))
            pzev = ctx.enter_context(tc.tile_pool(name="zev", bufs=3))
            pscp = ctx.enter_context(tc.tile_pool(name="scp", bufs=2))
            # PSUM pools: mm 3x2 banks + yt 1x2 banks = 8 banks
            pmm = ctx.enter_context(
                tc.tile_pool(name="mm", bufs=3, space="PSUM"))
            pyt = ctx.enter_context(
                tc.tile_pool(name="yt", bufs=1, space="PSUM"))

            # ---- DMA loads: batched into few big descriptors.
            # Compute-critical ones on the sync HWDGE queue, the rest on
            # the scalar HWDGE queue. Weight/x SBUF layout packs the 8
            # contraction chunks side by side in one tile so a single
            # 3D-AP DMA covers all of them.
            x_all = px.tile([P, KC * T], BF16, tag="x", name="x_all")
            w_all = {}
            for nm, dram in (("wv", wvT), ("wq", wqT), ("wk", wkT)):
                w_all[nm] = pw.tile([P, KC * HD], BF16, tag=nm, name=nm)
            t_x = [x_all[:, k * T:(k + 1) * T] for k in range(KC)]
            t_wv = [w_all["wv"][:, k * HD:(k + 1) * HD] for k in range(KC)]
            t_wq = [w_all["wq"][:, k * HD:(k + 1) * HD] for k in range(KC)]
            t_wk = [w_all["wk"][:, k * HD:(k + 1) * HD] for k in range(KC)]

            def dma_w(eng, tile, dram):
                # DRAM layout matches the SBUF tile -> one fat contiguous
                # transfer per partition
                eng.dma_start(tile[:], dram[:])

            def dma_x(eng, tb, c0=0, c1=TQB):
                dst = bass.AP(
                    x_all.tensor, x_all[:].offset + tb * TQB + c0,
                    [x_all[:].ap[0], [T, KC], [1, c1 - c0]],
                )
                src = xT[tb].rearrange(
                    "p (k c) -> p k c", k=KC)[:, :, c0:c1]
                eng.dma_start(dst, src)

            # critical-path loads first; the bulk x blocks and wp are
            # emitted later (after the first compute units) so their DMA
            # packets don't steal bandwidth from the blocks compute needs
            # right away
            # the first exp needs qk-proj of block 0 (wq/wk + x block 0);
            # prioritize those so ScalarE's stream starts early
            dma_w(nc.sync, w_all["wq"], wqT)
            dma_w(nc.sync, w_all["wk"], wkT)
            dma_x(nc.sync, 0, 0, TQB)
            dma_w(nc.sync, w_all["wv"], wvT)
            t_jte = pc.tile([P, 3 * P], BF16, tag="jte")
            nc.scalar.dma_start(t_jte[:], jte[:])
            t_j = t_jte[:, 0:P]
            t_tri = t_jte[:, P:2 * P]
            t_ea = t_jte[:, 2 * P:3 * P]
            t_bqkv = pc.tile([P, 4 * NJC + HD], F32, tag="bqkv")
            nc.scalar.dma_start(t_bqkv[:], bqkv[:])
            t_bq = t_bqkv[:, 0:NJC]
            t_bk = t_bqkv[:, NJC:2 * NJC]
            t_bqs = t_bqkv[:, 2 * NJC:3 * NJC]
            t_bks = t_bqkv[:, 3 * NJC:4 * NJC]
            t_bv = t_bqkv[:, 4 * NJC:4 * NJC + HD]
            t_trig = pc.tile([P, 2 * T], BF16, tag="trig")
            nc.scalar.dma_start(t_trig[:], trig[:])
            t_cos = t_trig[:, 0:T]
            t_sin = t_trig[:, T:2 * T]
            wp_all = pw.tile([P, NJC * C], BF16, tag="wp", name="wp_all")
            t_wp = [wp_all[:, jc * C:(jc + 1) * C] for jc in range(NJC)]

            def dma_bulk():
                for tb in range(1, NQB):
                    dma_x(nc.sync, tb)
                nc.sync.dma_start(wp_all[:], wpT[:])

            # softmax denominator staging (rows 0/64 carry data; the rest
            # must be finite zeros for the EA broadcast matmul)
            t_scp = [pscp.tile([P, TQB], BF16, tag=f"scp{i}", name=f"scp{i}")
                     for i in range(2)]
            nc.vector.memset(t_scp[0][:], 0.0)
            nc.vector.memset(t_scp[1][:], 0.0)

            # ---- persistent compute tiles ----
            t_v = [pv.tile([P, NJC * PW], BF16, tag=f"v{tt}", name=f"v{tt}")
                   for tt in range(NKC)]
            t_qrot = [pqk.tile([P, T], BF16, tag=f"qr{jc}", name=f"qrot{jc}")
                      for jc in range(NJC)]
            t_krot = [pqk.tile([P, T], BF16, tag=f"kr{jc}", name=f"krot{jc}")
                      for jc in range(NJC)]
            t_yn = [py.tile([P, T], BF16, tag=f"yn{jc}", name=f"yn{jc}")
                    for jc in range(NJC)]

            # ---- work units (emitted inline or interleaved into attention)
            def vproj_unit(tt):
                """v projection for t-block tt -> t_v[tt] [128, 2*130]."""
                vps = pmm.tile([P, 2 * TQB], F32, tag="mm")
                for k in range(KC):
                    nc.tensor.matmul(
                        vps[:, 0:HD],
                        lhsT=t_x[k][:, tt * P:(tt + 1) * P],
                        rhs=t_wv[k][:],
                        start=(k == 0),
                        stop=(k == KC - 1),
                    )
                # layout [128, 4*65]: head i at cols [i*65, i*65+64), a ones
                # column at i*65+64 (PV with it computes the softmax
                # denominator for free as an extra output row)
                v3 = t_v[tt][:].rearrange("p (h c) -> p h c", h=2 * NJC)
                nc.vector.tensor_add(
                    v3[:, :, 0:hd],
                    vps[:, 0:HD].rearrange("p (h c) -> p h c", h=2 * NJC),
                    t_bv[:].rearrange("p (h c) -> p h c", h=2 * NJC),
                )
                nc.gpsimd.memset(v3[:, :, hd:hd + 1], 1.0)

            def qkproj_unit(tb, jc, is_k):
                """q/k projection + RoPE for (pair jc, column block tb).
                RoPE reads the projection straight from PSUM; the pair-swap
                is a 64-partition base shift (legal because only one input
                is SBUF), so no J permutation matmul and no evacuation.
                rot = cos*(q+b) + sins*swap(q+b), with the swapped bias
                column provided by the host."""
                wchunks = t_wk if is_k else t_wq
                bias = t_bk if is_k else t_bq
                bias_s = t_bks if is_k else t_bqs
                dst = t_krot[jc] if is_k else t_qrot[jc]
                sl = slice(tb * TQB, (tb + 1) * TQB)
                A = mybir.AluOpType.add
                M = mybir.AluOpType.mult
                qps = pmm.tile([P, 2 * TQB], F32, tag="mm")
                for k in range(KC):
                    nc.tensor.matmul(
                        qps[:, 0:TQB],
                        lhsT=wchunks[k][:, jc * P:(jc + 1) * P],
                        rhs=t_x[k][:, sl],
                        start=(k == 0),
                        stop=(k == KC - 1),
                    )
                raw = praw.tile([P, TQB], BF16, tag="qkraw")
                nc.vector.tensor_scalar_add(
                    raw[:], qps[:, 0:TQB], bias[:, jc:jc + 1])
                # RoPE: rot = cos*raw + sins*(J@raw)
                jps = pmm.tile([P, 2 * TQB], F32, tag="mm")
                nc.tensor.matmul(jps[:, 0:TQB], lhsT=t_j[:], rhs=raw[:])
                tmp1 = prt.tile([P, TQB], BF16, tag="rope1")
                nc.vector.tensor_mul(tmp1[:], raw[:], t_cos[:, sl])
                tmp2 = prt.tile([P, TQB], BF16, tag="rope2")
                nc.vector.tensor_mul(tmp2[:], jps[:, 0:TQB], t_sin[:, sl])
                nc.vector.tensor_add(dst[:, sl], tmp1[:], tmp2[:])

            def outproj_unit(tt, on_scalar=False, pieces=None):
                """output projection + z DMA for t-block tt. With
                pieces, returns two ~0.4us closures (one per co chunk)
                for fine-grained interleave into ACT-bound blocks."""
                st = {}

                def co_piece(co):
                    if co == 0:
                        st["zps"] = pmm.tile([P, 2 * TQB], F32, tag="mm",
                                             name="zps")
                    zps = st["zps"]
                    for jc in range(NJC):
                        nc.tensor.matmul(
                            zps[:, co * TQB:(co + 1) * TQB],
                            lhsT=t_yn[jc][:, tt * P:(tt + 1) * P],
                            rhs=t_wp[jc][:, co * TQB:(co + 1) * TQB],
                            start=(jc == 0),
                            stop=(jc == NJC - 1),
                            skip_group_check=True,
                        )
                    if co == C // TQB - 1:
                        zev = pzev.tile([P, C], BF16, tag="zev")
                        if on_scalar:
                            nc.scalar.activation(
                                zev[:], zps[:],
                                mybir.ActivationFunctionType.Copy)
                        else:
                            nc.vector.tensor_copy(zev[:], zps[:])
                        nc.sync.dma_start(z[tt * P:(tt + 1) * P, :], zev[:])

                if pieces is not None:
                    pieces.append(lambda: co_piece(0))
                    pieces.append(lambda: co_piece(1))
                else:
                    co_piece(0)
                    co_piece(1)

            def attention_block(qb, units):
                """Causal attention for query block qb, interleaving the
                given list of work-unit closures into the kc loop.

                Software-pipelined: scores (+exp +mask) run LOOKAHEAD
                iterations ahead of the P@V consumers so the in-order PE
                queue never parks on an exp that hasn't run, and ScalarE
                (the pacer) always has a score tile to work on. The softmax
                finalize of each (qb, hp) phase is deferred until after the
                next phase's score prologue for the same reason."""
                n_kc = min(NKC, (qb + 1) * (TQB // P))
                n_iter = 2 * n_kc
                done = [0]

                def pop_units(idx):
                    # ceil so units a P@V depends on (v tiles, next-phase
                    # qk blocks) are emitted BEFORE their consumers
                    want = -((-(idx + 1) * len(units)) // n_iter)
                    while done[0] < min(want, len(units)):
                        units[done[0]]()
                        done[0] += 1

                LOOKAHEAD = 3
                it = 0
                for hp in range(NJC):
                    # per head: psum rows 0-63 = y, row 64 = denominator
                    yt_a = pyt.tile([P, TQB], F32, tag="yta")
                    yt_b = pyt.tile([P, TQB], F32, tag="ytb")
                    exq = {}

                    def emit_scores(kc, hp=hp, exq=exq):
                        s0 = max(0, kc * P - qb * TQB)
                        # scores for both heads of the pair -> one 2-bank tile
                        sc = pmm.tile([P, 2 * TQB], F32, tag="mm", name="sc")
                        for hl in range(2):
                            nc.tensor.matmul(
                                sc[:, hl * TQB + s0:(hl + 1) * TQB],
                                lhsT=t_krot[hp][
                                    hl * hd:(hl + 1) * hd,
                                    kc * P:(kc + 1) * P],
                                rhs=t_qrot[hp][
                                    hl * hd:(hl + 1) * hd,
                                    qb * TQB + s0:(qb + 1) * TQB],
                            )
                        # exp with 1/sqrt(hd) folded in; diag-trim left cols
                        ex = pexp.tile([P, 2 * TQB], BF16, tag="exp",
                                       name="ex")
                        sc3 = sc[:].rearrange("p (h w) -> p h w", h=2)
                        ex3 = ex[:].rearrange("p (h w) -> p h w", h=2)
                        nc.scalar.activation(
                            ex3[:, :, s0:TQB],
                            sc3[:, :, s0:TQB],
                            mybir.ActivationFunctionType.Exp,
                            scale=scale,
                        )
                        # diagonal 128-wide triangle mask (tk<=tq kept)
                        if kc * P >= qb * TQB:
                            tri3 = bass.AP(
                                t_tri.tensor, t_tri.offset,
                                [t_tri.ap[0], [0, 2], t_tri.ap[1]],
                            )
                            nc.gpsimd.tensor_mul(
                                ex3[:, :, s0:s0 + P],
                                ex3[:, :, s0:s0 + P],
                                tri3,
                            )
                        exq[kc] = ex

                    for kc in range(min(LOOKAHEAD, n_kc)):
                        emit_scores(kc)
                    fin = finalize_q.pop() if finalize_q else None
                    if fin is not None:
                        fin()
                    for kc in range(n_kc):
                        if kc + LOOKAHEAD < n_kc:
                            emit_scores(kc + LOOKAHEAD)
                        pop_units(it)
                        s0 = max(0, kc * P - qb * TQB)
                        ex = exq.pop(kc)
                        # P @ V; ones columns produce the denominators
                        i0, i1 = 2 * hp, 2 * hp + 1
                        nc.tensor.matmul(
                            yt_a[0:VW, s0:TQB],
                            lhsT=t_v[kc][:, i0 * VW:(i0 + 1) * VW],
                            rhs=ex[:, s0:TQB],
                            start=(kc == 0),
                            stop=(kc == n_kc - 1),
                            skip_group_check=True,
                        )
                        nc.tensor.matmul(
                            yt_b[0:VW, s0:TQB],
                            lhsT=t_v[kc][:, i1 * VW:(i1 + 1) * VW],
                            rhs=ex[:, TQB + s0:2 * TQB],
                            start=(kc == 0),
                            stop=(kc == n_kc - 1),
                            skip_group_check=True,
                        )
                        it += 1

                    def finalize(hp=hp, yt_a=yt_a, yt_b=yt_b):
                        # stage both denominator rows, broadcast via EA
                        # matmul, reciprocal on VectorE, scale into t_yn
                        scp = t_scp[hp]
                        with nc.allow_low_precision(reason="bf16 denom"):
                            nc.vector.tensor_copy(
                                scp[0:1, :], yt_a[hd:hd + 1, :])
                            nc.vector.tensor_copy(
                                scp[hd:hd + 1, :], yt_b[hd:hd + 1, :])
                        bc = pmm.tile([P, 2 * TQB], F32, tag="mm", name="bc")
                        nc.tensor.matmul(
                            bc[:, 0:TQB], lhsT=t_ea[:], rhs=scp[:])
                        rcpb = prcp.tile([P, TQB], F32, tag="rcpb")
                        nc.vector.reciprocal_approx_fast(rcpb[:], bc[:, 0:TQB])
                        nc.vector.tensor_mul(
                            t_yn[hp][0:hd, qb * TQB:(qb + 1) * TQB],
                            yt_a[0:hd, :], rcpb[0:hd, :])
                        nc.vector.tensor_mul(
                            t_yn[hp][hd:2 * hd, qb * TQB:(qb + 1) * TQB],
                            yt_b[0:hd, :], rcpb[hd:2 * hd, :])

                    finalize_q.append(finalize)

            # ---- schedule ----
            finalize_q = []

            def qk_units(tb, jcs=range(NJC)):
                return [
                    lambda tb=tb, jc=jc, k=is_k: qkproj_unit(tb, jc, k)
                    for jc in jcs for is_k in (False, True)]

            def v_units(tb):
                return [lambda tt=tb * (TQB // P) + i: vproj_unit(tt)
                        for i in range(TQB // P)]

            def proj_units(qb, lo=0, hi=TQB // P, on_scalar=False,
                           split=False):
                if not split:
                    return [
                        lambda tt=qb * (TQB // P) + i:
                        outproj_unit(tt, on_scalar)
                        for i in range(lo, hi)]
                ps = []
                for i in range(lo, hi):
                    outproj_unit(qb * (TQB // P) + i, on_scalar, pieces=ps)
                return ps

            # prologue: the qk projection the first scores need comes
            # first so ScalarE starts early; v(0) follows (its consumers,
            # the P@Vs, trail the scores by LOOKAHEAD iterations)
            for u in qk_units(0, [0]):
                u()
            dma_bulk()
            for u in v_units(0):
                u()
            attention_block(0, qk_units(0, [1]) + qk_units(1) + v_units(1))
            attention_block(1, _ilv(proj_units(0, 0, 2),
                                    qk_units(2) + v_units(2)))
            attention_block(2, _ilv(proj_units(0, 2, 4) + proj_units(1, 0, 2),
                                    qk_units(3) + v_units(3)))
            attention_block(3, proj_units(1, 2, 4) + proj_units(2))
            while finalize_q:
                finalize_q.pop()()
            for u in proj_units(3, on_scalar=True):
                u()


def _ilv(a, b):
    """Interleave two unit lists: a0 b0 b1 a1 b2 b3 ..."""
    out = []
    ia = ib = 0
    while ia < len(a) or ib < len(b):
        if ia < len(a):
            out.append(a[ia])
            ia += 1
        for _ in range(2):
            if ib < len(b):
                out.append(b[ib])
                ib += 1
    return out


_ROPE_PERM = np.concatenate([np.arange(0, 64, 2), np.arange(1, 64, 2)])


def _host_inputs(x_b, Wq, bq, Wk, bk, Wv, bv, Wp, heads, T, C, hd):
    """Build the per-core DRAM input dict (numpy)."""
    HD = len(heads) * hd
    rows = np.concatenate([h * hd + _ROPE_PERM for h in heads])
    rows_nop = np.concatenate([np.arange(h * hd, (h + 1) * hd) for h in heads])

    KC, NQB, TQB = C // P, T // 512, 512
    xT = np.ascontiguousarray(
        x_b.T.reshape(KC, P, NQB, TQB).transpose(2, 1, 0, 3)
        .reshape(NQB, P, KC * TQB)).astype(NPBF16)

    def _wmaj(w):  # [C, M] -> [P, KC*M] partition-major chunk layout
        m = w.shape[1]
        return np.ascontiguousarray(
            w.reshape(KC, P, m).transpose(1, 0, 2).reshape(P, KC * m))

    wqT = _wmaj(Wq[rows].T).astype(NPBF16)
    wkT = _wmaj(Wk[rows].T).astype(NPBF16)
    wvT = _wmaj(Wv[rows_nop].T).astype(NPBF16)
    wpT = np.ascontiguousarray(
        Wp[:, rows_nop].T.reshape(HD // P, P, C).transpose(1, 0, 2)
        .reshape(P, (HD // P) * C)).astype(NPBF16)

    j = np.arange(hd // 2, dtype=np.float64)
    inv_freq = 1.0 / (10000.0 ** (2.0 * j / hd))
    t = np.arange(T, dtype=np.float64)
    ang = t[:, None] * inv_freq[None, :]          # [T, 32]
    cos = np.cos(ang)
    sin = np.sin(ang)
    r = np.arange(P)
    cosq = cos[:, r % (hd // 2)].T.astype(np.float32)
    sgn = np.where((r % hd) < hd // 2, -1.0, 1.0)
    sinsq = (sin[:, r % (hd // 2)] * sgn[None, :]).T.astype(np.float32)

    pair = np.where((r % hd) < hd // 2, r + hd // 2, r - hd // 2)
    jmat = np.zeros((P, P), np.float32)
    jmat[pair, r] = 1.0
    tri = (np.arange(P)[None, :] >= np.arange(P)[:, None]).astype(np.float32)
    ea = np.zeros((P, P), np.float32)
    ea[(r // hd) * hd, r] = 1.0

    bqTh = bq[rows].reshape(HD // P, P).T.astype(np.float32)
    bkTh = bk[rows].reshape(HD // P, P).T.astype(np.float32)
    bvb = np.tile(bv[rows_nop][None, :], (P, 1)).astype(np.float32)

    return {
        "xT": xT, "wqT": wqT, "wkT": wkT, "wvT": wvT, "wpT": wpT,
        "trig": np.ascontiguousarray(
            np.concatenate([cosq, sinsq], axis=1)).astype(NPBF16),
        "jte": np.ascontiguousarray(
            np.concatenate([jmat, tri, ea], axis=1)).astype(NPBF16),
        "bqkv": np.ascontiguousarray(np.concatenate(
            [bqTh, bkTh, bqTh[pair], bkTh[pair], bvb],
            axis=1)).astype(np.float32),
    }


def make_core_inputs(x, Wq, bq, Wk, bk, Wv, bv, Wp, T=2048, C=1024, hd=64,
                     heads_per_core=4):
    in_maps = []
    for c in range(N_CORES):
        b = c // 4
        g = c % 4
        heads = list(range(g * heads_per_core, (g + 1) * heads_per_core))
        in_maps.append(_host_inputs(
            np.asarray(x[b]), Wq, bq, Wk, bk, Wv, bv, Wp, heads, T, C, hd))
    return in_maps


def kernel(x, Wq, bq, Wk, bk, Wv, bv, Wp, bp):
    x = np.asarray(x, np.float32)
    Wq = np.asarray(Wq, np.float32)
    bq = np.asarray(bq, np.float32)
    Wk = np.asarray(Wk, np.float32)
    bk = np.asarray(bk, np.float32)
    Wv = np.asarray(Wv, np.float32)
    bv = np.asarray(bv, np.float32)
    Wp = np.asarray(Wp, np.float32)
    bp = np.asarray(bp, np.float32)
    B, T, C = x.shape

    _patch_act_tables()
    nc = bacc.Bacc("TRN2", target_bir_lowering=False, debug=False,
                   num_devices=N_CORES)
    build_attention_kernel(nc, T=T, C=C)
    nc.compile()

    in_maps = make_core_inputs(x, Wq, bq, Wk, bk, Wv, bv, Wp, T=T, C=C)
    res = run_bass_kernel_spmd(nc, in_maps, list(range(N_CORES)))

    out = np.zeros((B, T, C), np.float32)
    for c in range(N_CORES):
        out[c // 4] += res.results[c]["z"]
    out += bp[None, None, :]
    return out


if __name__ == "__main__":
    import reference

    inputs = reference.setup_inputs()
    expected = np.asarray(reference.reference(**inputs))
    actual = kernel(**{k: np.asarray(v) for k, v in inputs.items()})
    err = np.abs(actual - expected).max() / np.abs(expected).max()
    print("Relative error:", err)


# revision 57
# speedup vs baseline: 1.2027x; 1.2027x over previous
"""Causal self-attention (B=2, T=2048, C=1024, H=16, RoPE) on 8 TRN2 cores.

Sharding: data-parallel over B (2 groups of 4 cores) x tensor-parallel over
heads (4 heads per core). Each core computes q/k/v projections for its heads,
RoPE, causal attention, and its partial output projection; the host sums the
4 partial projections per batch and adds bp.

Layout choices (per core):
  - x and weights arrive host-pre-tiled so every SBUF tile is a contiguous
    DRAM block (few, fat DMA descriptors); x lands in four 512-column
    blocks so projections start before the full load arrives.
  - q, k produced TRANSPOSED: qT/kT [256=4heads*64, T] via lhsT=W, rhs=xT.
    Head-dim pairs are pre-permuted (evens|odds) in the weights so RoPE
    needs no strided access; the pair-swap is a constant permutation
    matmul (J), combine on VectorE in bf16 (packed 2x mode).
  - v produced NON-transposed, per-head layout [v|1] (65 cols/head): the
    ones column rides along in each P@V matmul and produces the softmax
    denominator for free (psum row 64, no extra reduction).
  - scores computed transposed: ST[tk, tq] = k_rot @ q_rot^T per head; the
    two heads of a pair sit on partitions 0-63/64-127, so their score
    matmuls run CONCURRENTLY on the PE's 64x128 row tiles. Softmax-exp is
    elementwise (ScalarE, scale=1/8 folded in), the causal mask is a fixed
    128x128 triangle on diagonal blocks (GpSimd), fully-masked blocks are
    skipped, and diagonal blocks are column-trimmed in the score matmul,
    exp, and P@V (free dim starts at the diagonal).
  - softmax reciprocal on VectorE (reciprocal_approx_fast), broadcast over
    partitions with a constant matmul (EA).
  - the whole kernel is software-pipelined around ScalarE's exp stream
    (the steady-state pacer): scores+exp run LOOKAHEAD iterations ahead of
    their P@V consumers, each phase's softmax finalize is deferred past
    the next phase's score prologue, and projection work units (next x
    block's qkv proj, previous block's output proj) are interleaved into
    the attention kc loops to fill the PE's per-iteration slack.
"""

import math

import numpy as np
import ml_dtypes

import concourse.bass as bass
import concourse.bacc as bacc
import concourse.mybir as mybir
from concourse.tile import TileContext
from concourse.bass_utils import run_bass_kernel_spmd

BF16 = mybir.dt.bfloat16
F32 = mybir.dt.float32
NPBF16 = ml_dtypes.bfloat16

N_CORES = 8
P = 128

_UNIFIED_ACT_SET = "natural_log_exp_and_others"


def _patch_act_tables():
    import concourse.hw_specs as _hw
    import concourse.bacc as _bacc
    if getattr(_bacc, "_act_tables_patched", False):
        return
    _orig = _hw.get_activation_tables

    def _gat(arch):
        tabs = _orig(arch)
        if _UNIFIED_ACT_SET in tabs:
            keep = tabs[_UNIFIED_ACT_SET]
            drop = {
                mybir.ActivationFunctionType.Exp,
                mybir.ActivationFunctionType.Copy,
            } & keep
            for name, fns in tabs.items():
                if name != _UNIFIED_ACT_SET:
                    for f in drop:
                        fns.discard(f)
        return tabs

    _bacc.get_activation_tables = _gat
    _bacc._act_tables_patched = True


def build_attention_kernel(nc, T=2048, C=1024, n_heads=4, hd=64):
    """Emit the per-core kernel. Returns nothing; tensors are declared on nc."""
    _patch_act_tables()
    HD = n_heads * hd            # 256: local head dims
    KC = C // P                  # 8: contraction chunks for projections
    NJC = HD // P                # 2: partition tiles of qT/kT (head pairs)
    TQB = 512                    # tq block for scores/PV
    NQB = T // TQB               # 4
    NKC = T // P                 # 16: tk chunks
    VW = hd + 1                  # 65: v cols per head incl ones column
    PW = 2 * VW                  # 130: v cols per head pair
    scale = 1.0 / math.sqrt(hd)

    # ---- DRAM I/O ----
    # x and the weights arrive pre-tiled so every SBUF tile is one
    # contiguous DRAM block (big DMA descriptors, cheap triggers)
    xT = nc.declare_dram_parameter("xT", [NQB, P, KC * TQB], BF16,
                                   isOutput=False)
    wqT = nc.declare_dram_parameter("wqT", [P, KC * HD], BF16, isOutput=False)
    wkT = nc.declare_dram_parameter("wkT", [P, KC * HD], BF16, isOutput=False)
    wvT = nc.declare_dram_parameter("wvT", [P, KC * HD], BF16, isOutput=False)
    wpT = nc.declare_dram_parameter("wpT", [P, NJC * C], BF16, isOutput=False)
    # trig = [cos | sin], jte = [J | tri | EA], bqkv = [bq | bk | bv_row]
    trig = nc.declare_dram_parameter("trig", [P, 2 * T], BF16, isOutput=False)
    jte = nc.declare_dram_parameter("jte", [P, 3 * P], BF16, isOutput=False)
    bqkv = nc.declare_dram_parameter("bqkv", [P, 4 * NJC + HD], F32,
                                     isOutput=False)
    z = nc.declare_dram_parameter("z", [T, C], BF16, isOutput=True)

    with TileContext(nc) as tc:
        import contextlib

        with contextlib.ExitStack() as ctx:
            # ---- persistent SBUF pools ----
            pc = ctx.enter_context(tc.tile_pool(name="const", bufs=1))
            px = ctx.enter_context(tc.tile_pool(name="x", bufs=1))
            pw = ctx.enter_context(tc.tile_pool(name="w", bufs=1))
            pqk = ctx.enter_context(tc.tile_pool(name="qk", bufs=1))
            pv = ctx.enter_context(tc.tile_pool(name="v", bufs=1))
            py = ctx.enter_context(tc.tile_pool(name="y", bufs=1))
            # transient pools
            praw = ctx.enter_context(tc.tile_pool(name="raw", bufs=2))
            prt = ctx.enter_context(tc.tile_pool(name="ropetmp", bufs=4))
            pexp = ctx.enter_context(tc.tile_pool(name="exp", bufs=6))
            prcp = ctx.enter_context(tc.tile_pool(name="rcp", bufs=3))
            pzev = ctx.enter_context(tc.tile_pool(name="zev", bufs=3))
            pscp = ctx.enter_context(tc.tile_pool(name="scp", bufs=2))
            # PSUM pools: mm 3x2 banks + yt 1x2 banks = 8 banks
            pmm = ctx.enter_context(
                tc.tile_pool(name="mm", bufs=3, space="PSUM"))
            pyt = ctx.enter_context(
                tc.tile_pool(name="yt", bufs=1, space="PSUM"))

            # ---- DMA loads: batched into few big descriptors.
            # Compute-critical ones on the sync HWDGE queue, the rest on
            # the scalar HWDGE queue. Weight/x SBUF layout packs the 8
            # contraction chunks side by side in one tile so a single
            # 3D-AP DMA covers all of them.
            x_all = px.tile([P, KC * T], BF16, tag="x", name="x_all")
            w_all = {}
            for nm, dram in (("wv", wvT), ("wq", wqT), ("wk", wkT)):
                w_all[nm] = pw.tile([P, KC * HD], BF16, tag=nm, name=nm)
            t_x = [x_all[:, k * T:(k + 1) * T] for k in range(KC)]
            t_wv = [w_all["wv"][:, k * HD:(k + 1) * HD] for k in range(KC)]
            t_wq = [w_all["wq"][:, k * HD:(k + 1) * HD] for k in range(KC)]
            t_wk = [w_all["wk"][:, k * HD:(k + 1) * HD] for k in range(KC)]

            def dma_w(eng, tile, dram):
                # DRAM layout matches the SBUF tile -> one fat contiguous
                # transfer per partition
                eng.dma_start(tile[:], dram[:])

            def dma_x(eng, tb, c0=0, c1=TQB):
                dst = bass.AP(
                    x_all.tensor, x_all[:].offset + tb * TQB + c0,
                    [x_all[:].ap[0], [T, KC], [1, c1 - c0]],
                )
                src = xT[tb].rearrange(
                    "p (k c) -> p k c", k=KC)[:, :, c0:c1]
                eng.dma_start(dst, src)

            # critical-path loads first; the bulk x blocks and wp are
            # emitted later (after the first compute units) so their DMA
            # packets don't steal bandwidth from the blocks compute needs
            # right away
            # the first exp needs qk-proj of block 0 (wq/wk + x block 0);
            # prioritize those AND split them across BOTH HWDGE rings
            # (sync + scalar) so descriptor generation and transfer flow
            # run in parallel
            dma_w(nc.sync, w_all["wq"], wqT)
            dma_w(nc.scalar, w_all["wk"], wkT)
            dma_x(nc.sync, 0, 0, TQB // 2)
            dma_x(nc.scalar, 0, TQB // 2, TQB)
            dma_w(nc.sync, w_all["wv"], wvT)
            t_jte = pc.tile([P, 3 * P], BF16, tag="jte")
            nc.scalar.dma_start(t_jte[:], jte[:])
            t_j = t_jte[:, 0:P]
            t_tri = t_jte[:, P:2 * P]
            t_ea = t_jte[:, 2 * P:3 * P]
            t_bqkv = pc.tile([P, 4 * NJC + HD], F32, tag="bqkv")
            nc.scalar.dma_start(t_bqkv[:], bqkv[:])
            t_bq = t_bqkv[:, 0:NJC]
            t_bk = t_bqkv[:, NJC:2 * NJC]
            t_bqs = t_bqkv[:, 2 * NJC:3 * NJC]
            t_bks = t_bqkv[:, 3 * NJC:4 * NJC]
            t_bv = t_bqkv[:, 4 * NJC:4 * NJC + HD]
            t_trig = pc.tile([P, 2 * T], BF16, tag="trig")
            nc.scalar.dma_start(t_trig[:], trig[:])
            t_cos = t_trig[:, 0:T]
            t_sin = t_trig[:, T:2 * T]
            wp_all = pw.tile([P, NJC * C], BF16, tag="wp", name="wp_all")
            t_wp = [wp_all[:, jc * C:(jc + 1) * C] for jc in range(NJC)]

            def dma_bulk():
                for tb in range(1, NQB):
                    dma_x(nc.sync, tb)
                nc.sync.dma_start(wp_all[:], wpT[:])

            # softmax denominator staging (rows 0/64 carry data; the rest
            # must be finite zeros for the EA broadcast matmul)
            t_scp = [pscp.tile([P, TQB], BF16, tag=f"scp{i}", name=f"scp{i}")
                     for i in range(2)]
            nc.vector.memset(t_scp[0][:], 0.0)
            nc.vector.memset(t_scp[1][:], 0.0)

            # ---- persistent compute tiles ----
            t_v = [pv.tile([P, NJC * PW], BF16, tag=f"v{tt}", name=f"v{tt}")
                   for tt in range(NKC)]
            t_qrot = [pqk.tile([P, T], BF16, tag=f"qr{jc}", name=f"qrot{jc}")
                      for jc in range(NJC)]
            t_krot = [pqk.tile([P, T], BF16, tag=f"kr{jc}", name=f"krot{jc}")
                      for jc in range(NJC)]
            t_yn = [py.tile([P, T], BF16, tag=f"yn{jc}", name=f"yn{jc}")
                    for jc in range(NJC)]

            # ---- work units (emitted inline or interleaved into attention)
            def vproj_unit(tt):
                """v projection for t-block tt -> t_v[tt] [128, 2*130]."""
                vps = pmm.tile([P, 2 * TQB], F32, tag="mm")
                for k in range(KC):
                    nc.tensor.matmul(
                        vps[:, 0:HD],
                        lhsT=t_x[k][:, tt * P:(tt + 1) * P],
                        rhs=t_wv[k][:],
                        start=(k == 0),
                        stop=(k == KC - 1),
                    )
                # layout [128, 4*65]: head i at cols [i*65, i*65+64), a ones
                # column at i*65+64 (PV with it computes the softmax
                # denominator for free as an extra output row)
                v3 = t_v[tt][:].rearrange("p (h c) -> p h c", h=2 * NJC)
                nc.vector.tensor_add(
                    v3[:, :, 0:hd],
                    vps[:, 0:HD].rearrange("p (h c) -> p h c", h=2 * NJC),
                    t_bv[:].rearrange("p (h c) -> p h c", h=2 * NJC),
                )
                nc.gpsimd.memset(v3[:, :, hd:hd + 1], 1.0)

            def qkproj_unit(tb, jc, is_k):
                """q/k projection + RoPE for (pair jc, column block tb).
                RoPE reads the projection straight from PSUM; the pair-swap
                is a 64-partition base shift (legal because only one input
                is SBUF), so no J permutation matmul and no evacuation.
                rot = cos*(q+b) + sins*swap(q+b), with the swapped bias
                column provided by the host."""
                wchunks = t_wk if is_k else t_wq
                bias = t_bk if is_k else t_bq
                bias_s = t_bks if is_k else t_bqs
                dst = t_krot[jc] if is_k else t_qrot[jc]
                sl = slice(tb * TQB, (tb + 1) * TQB)
                A = mybir.AluOpType.add
                M = mybir.AluOpType.mult
                qps = pmm.tile([P, 2 * TQB], F32, tag="mm")
                for k in range(KC):
                    nc.tensor.matmul(
                        qps[:, 0:TQB],
                        lhsT=wchunks[k][:, jc * P:(jc + 1) * P],
                        rhs=t_x[k][:, sl],
                        start=(k == 0),
                        stop=(k == KC - 1),
                    )
                raw = praw.tile([P, TQB], BF16, tag="qkraw")
                nc.vector.tensor_scalar_add(
                    raw[:], qps[:, 0:TQB], bias[:, jc:jc + 1])
                # RoPE: rot = cos*raw + sins*(J@raw)
                jps = pmm.tile([P, 2 * TQB], F32, tag="mm")
                nc.tensor.matmul(jps[:, 0:TQB], lhsT=t_j[:], rhs=raw[:])
                tmp1 = prt.tile([P, TQB], BF16, tag="rope1")
                nc.vector.tensor_mul(tmp1[:], raw[:], t_cos[:, sl])
                tmp2 = prt.tile([P, TQB], BF16, tag="rope2")
                nc.vector.tensor_mul(tmp2[:], jps[:, 0:TQB], t_sin[:, sl])
                nc.vector.tensor_add(dst[:, sl], tmp1[:], tmp2[:])

            def outproj_unit(tt, on_scalar=False, pieces=None):
                """output projection + z DMA for t-block tt. With
                pieces, returns two ~0.4us closures (one per co chunk)
                for fine-grained interleave into ACT-bound blocks."""
                st = {}

                def co_piece(co):
                    if co == 0:
                        st["zps"] = pmm.tile([P, 2 * TQB], F32, tag="mm",
                                             name="zps")
                    zps = st["zps"]
                    for jc in range(NJC):
                        nc.tensor.matmul(
                            zps[:, co * TQB:(co + 1) * TQB],
                            lhsT=t_yn[jc][:, tt * P:(tt + 1) * P],
                            rhs=t_wp[jc][:, co * TQB:(co + 1) * TQB],
                            start=(jc == 0),
                            stop=(jc == NJC - 1),
                            skip_group_check=True,
                        )
                    if co == C // TQB - 1:
                        zev = pzev.tile([P, C], BF16, tag="zev")
                        if on_scalar:
                            nc.scalar.activation(
                                zev[:], zps[:],
                                mybir.ActivationFunctionType.Copy)
                        else:
                            nc.vector.tensor_copy(zev[:], zps[:])
                        nc.sync.dma_start(z[tt * P:(tt + 1) * P, :], zev[:])

                if pieces is not None:
                    pieces.append(lambda: co_piece(0))
                    pieces.append(lambda: co_piece(1))
                else:
                    co_piece(0)
                    co_piece(1)

            def attention_block(qb, units):
                """Causal attention for query block qb, interleaving the
                given list of work-unit closures into the kc loop.

                Software-pipelined: scores (+exp +mask) run LOOKAHEAD
                iterations ahead of the P@V consumers so the in-order PE
                queue never parks on an exp that hasn't run, and ScalarE
                (the pacer) always has a score tile to work on. The softmax
                finalize of each (qb, hp) phase is deferred until after the
                next phase's score prologue for the same reason."""
                n_kc = min(NKC, (qb + 1) * (TQB // P))
                n_iter = 2 * n_kc
                done = [0]

                def pop_units(idx):
                    # ceil so units a P@V depends on (v tiles, next-phase
                    # qk blocks) are emitted BEFORE their consumers
                    want = -((-(idx + 1) * len(units)) // n_iter)
                    while done[0] < min(want, len(units)):
                        units[done[0]]()
                        done[0] += 1

                LOOKAHEAD = 3
                it = 0
                for hp in range(NJC):
                    # per head: psum rows 0-63 = y, row 64 = denominator
                    yt_a = pyt.tile([P, TQB], F32, tag="yta")
                    yt_b = pyt.tile([P, TQB], F32, tag="ytb")
                    exq = {}

                    def emit_scores(kc, hp=hp, exq=exq):
                        s0 = max(0, kc * P - qb * TQB)
                        # scores for both heads of the pair -> one 2-bank tile
                        sc = pmm.tile([P, 2 * TQB], F32, tag="mm", name="sc")
                        for hl in range(2):
                            nc.tensor.matmul(
                                sc[:, hl * TQB + s0:(hl + 1) * TQB],
                                lhsT=t_krot[hp][
                                    hl * hd:(hl + 1) * hd,
                                    kc * P:(kc + 1) * P],
                                rhs=t_qrot[hp][
                                    hl * hd:(hl + 1) * hd,
                                    qb * TQB + s0:(qb + 1) * TQB],
                            )
                        # exp with 1/sqrt(hd) folded in; diag-trim left cols
                        ex = pexp.tile([P, 2 * TQB], BF16, tag="exp",
                                       name="ex")
                        sc3 = sc[:].rearrange("p (h w) -> p h w", h=2)
                        ex3 = ex[:].rearrange("p (h w) -> p h w", h=2)
                        nc.scalar.activation(
                            ex3[:, :, s0:TQB],
                            sc3[:, :, s0:TQB],
                            mybir.ActivationFunctionType.Exp,
                            scale=scale,
                        )
                        # diagonal 128-wide triangle mask (tk<=tq kept)
                        if kc * P >= qb * TQB:
                            tri3 = bass.AP(
                                t_tri.tensor, t_tri.offset,
                                [t_tri.ap[0], [0, 2], t_tri.ap[1]],
                            )
                            nc.gpsimd.tensor_mul(
                                ex3[:, :, s0:s0 + P],
                                ex3[:, :, s0:s0 + P],
                                tri3,
                            )
                        exq[kc] = ex

                    for kc in range(min(LOOKAHEAD, n_kc)):
                        emit_scores(kc)
                    fin = finalize_q.pop() if finalize_q else None
                    if fin is not None:
                        fin()
                    for kc in range(n_kc):
                        if kc + LOOKAHEAD < n_kc:
                            emit_scores(kc + LOOKAHEAD)
                        pop_units(it)
                        s0 = max(0, kc * P - qb * TQB)
                        ex = exq.pop(kc)
                        # P @ V; ones columns produce the denominators
                        i0, i1 = 2 * hp, 2 * hp + 1
                        nc.tensor.matmul(
                            yt_a[0:VW, s0:TQB],
                            lhsT=t_v[kc][:, i0 * VW:(i0 + 1) * VW],
                            rhs=ex[:, s0:TQB],
                            start=(kc == 0),
                            stop=(kc == n_kc - 1),
                            skip_group_check=True,
                        )
                        nc.tensor.matmul(
                            yt_b[0:VW, s0:TQB],
                            lhsT=t_v[kc][:, i1 * VW:(i1 + 1) * VW],
                            rhs=ex[:, TQB + s0:2 * TQB],
                            start=(kc == 0),
                            stop=(kc == n_kc - 1),
                            skip_group_check=True,
                        )
                        it += 1

                    def finalize(hp=hp, yt_a=yt_a, yt_b=yt_b):
                        # stage both denominator rows, broadcast via EA
                        # matmul, reciprocal on VectorE, scale into t_yn
                        scp = t_scp[hp]
                        with nc.allow_low_precision(reason="bf16 denom"):
                            nc.vector.tensor_copy(
                                scp[0:1, :], yt_a[hd:hd + 1, :])
                            nc.vector.tensor_copy(
                                scp[hd:hd + 1, :], yt_b[hd:hd + 1, :])
                        bc = pmm.tile([P, 2 * TQB], F32, tag="mm", name="bc")
                        nc.tensor.matmul(
                            bc[:, 0:TQB], lhsT=t_ea[:], rhs=scp[:])
                        rcpb = prcp.tile([P, TQB], F32, tag="rcpb")
                        nc.vector.reciprocal_approx_fast(rcpb[:], bc[:, 0:TQB])
                        nc.vector.tensor_mul(
                            t_yn[hp][0:hd, qb * TQB:(qb + 1) * TQB],
                            yt_a[0:hd, :], rcpb[0:hd, :])
                        nc.vector.tensor_mul(
                            t_yn[hp][hd:2 * hd, qb * TQB:(qb + 1) * TQB],
                            yt_b[0:hd, :], rcpb[hd:2 * hd, :])

                    finalize_q.append(finalize)

            # ---- schedule ----
            finalize_q = []

            def qk_units(tb, jcs=range(NJC)):
                return [
                    lambda tb=tb, jc=jc, k=is_k: qkproj_unit(tb, jc, k)
                    for jc in jcs for is_k in (False, True)]

            def v_units(tb):
                return [lambda tt=tb * (TQB // P) + i: vproj_unit(tt)
                        for i in range(TQB // P)]

            def proj_units(qb, lo=0, hi=TQB // P, on_scalar=False,
                           split=False):
                if not split:
                    return [
                        lambda tt=qb * (TQB // P) + i:
                        outproj_unit(tt, on_scalar)
                        for i in range(lo, hi)]
                ps = []
                for i in range(lo, hi):
                    outproj_unit(qb * (TQB // P) + i, on_scalar, pieces=ps)
                return ps

            # prologue: the qk projection the first scores need comes
            # first so ScalarE starts early; v(0) follows (its consumers,
            # the P@Vs, trail the scores by LOOKAHEAD iterations)
            for u in qk_units(0, [0]):
                u()
            dma_bulk()
            for u in v_units(0):
                u()
            attention_block(0, qk_units(0, [1]) + qk_units(1) + v_units(1))
            attention_block(1, _ilv(proj_units(0), qk_units(2) + v_units(2)))
            attention_block(2, _ilv(proj_units(1, 0, 2),
                                    qk_units(3) + v_units(3)))
            attention_block(3, proj_units(1, 2, 4) + proj_units(2))
            while finalize_q:
                finalize_q.pop()()
            for u in proj_units(3, on_scalar=True):
                u()


def _ilv(a, b):
    """Interleave two unit lists: a0 b0 b1 a1 b2 b3 ..."""
    out = []
    ia = ib = 0
    while ia < len(a) or ib < len(b):
        if ia < len(a):
            out.append(a[ia])
            ia += 1
        for _ in range(2):
            if ib < len(b):
                out.append(b[ib])
                ib += 1
    return out


_ROPE_PERM = np.concatenate([np.arange(0, 64, 2), np.arange(1, 64, 2)])


def _host_inputs(x_b, Wq, bq, Wk, bk, Wv, bv, Wp, heads, T, C, hd):
    """Build the per-core DRAM input dict (numpy)."""
    HD = len(heads) * hd
    rows = np.concatenate([h * hd + _ROPE_PERM for h in heads])
    rows_nop = np.concatenate([np.arange(h * hd, (h + 1) * hd) for h in heads])

    KC, NQB, TQB = C // P, T // 512, 512
    xT = np.ascontiguousarray(
        x_b.T.reshape(KC, P, NQB, TQB).transpose(2, 1, 0, 3)
        .reshape(NQB, P, KC * TQB)).astype(NPBF16)

    def _wmaj(w):  # [C, M] -> [P, KC*M] partition-major chunk layout
        m = w.shape[1]
        return np.ascontiguousarray(
            w.reshape(KC, P, m).transpose(1, 0, 2).reshape(P, KC * m))

    wqT = _wmaj(Wq[rows].T).astype(NPBF16)
    wkT = _wmaj(Wk[rows].T).astype(NPBF16)
    wvT = _wmaj(Wv[rows_nop].T).astype(NPBF16)
    wpT = np.ascontiguousarray(
        Wp[:, rows_nop].T.reshape(HD // P, P, C).transpose(1, 0, 2)
        .reshape(P, (HD // P) * C)).astype(NPBF16)

    j = np.arange(hd // 2, dtype=np.float64)
    inv_freq = 1.0 / (10000.0 ** (2.0 * j / hd))
    t = np.arange(T, dtype=np.float64)
    ang = t[:, None] * inv_freq[None, :]          # [T, 32]
    cos = np.cos(ang)
    sin = np.sin(ang)
    r = np.arange(P)
    cosq = cos[:, r % (hd // 2)].T.astype(np.float32)
    sgn = np.where((r % hd) < hd // 2, -1.0, 1.0)
    sinsq = (sin[:, r % (hd // 2)] * sgn[None, :]).T.astype(np.float32)

    pair = np.where((r % hd) < hd // 2, r + hd // 2, r - hd // 2)
    jmat = np.zeros((P, P), np.float32)
    jmat[pair, r] = 1.0
    tri = (np.arange(P)[None, :] >= np.arange(P)[:, None]).astype(np.float32)
    ea = np.zeros((P, P), np.float32)
    ea[(r // hd) * hd, r] = 1.0

    bqTh = bq[rows].reshape(HD // P, P).T.astype(np.float32)
    bkTh = bk[rows].reshape(HD // P, P).T.astype(np.float32)
    bvb = np.tile(bv[rows_nop][None, :], (P, 1)).astype(np.float32)

    return {
        "xT": xT, "wqT": wqT, "wkT": wkT, "wvT": wvT, "wpT": wpT,
        "trig": np.ascontiguousarray(
            np.concatenate([cosq, sinsq], axis=1)).astype(NPBF16),
        "jte": np.ascontiguousarray(
            np.concatenate([jmat, tri, ea], axis=1)).astype(NPBF16),
        "bqkv": np.ascontiguousarray(np.concatenate(
            [bqTh, bkTh, bqTh[pair], bkTh[pair], bvb],
            axis=1)).astype(np.float32),
    }


def make_core_inputs(x, Wq, bq, Wk, bk, Wv, bv, Wp, T=2048, C=1024, hd=64,
                     heads_per_core=4):
    in_maps = []
    for c in range(N_CORES):
        b = c // 4
        g = c % 4
        heads = list(range(g * heads_per_core, (g + 1) * heads_per_core))
        in_maps.append(_host_inputs(
            np.asarray(x[b]), Wq, bq, Wk, bk, Wv, bv, Wp, heads, T, C, hd))
    return in_maps


def kernel(x, Wq, bq, Wk, bk, Wv, bv, Wp, bp):
    x = np.asarray(x, np.float32)
    Wq = np.asarray(Wq, np.float32)
    bq = np.asarray(bq, np.float32)
    Wk = np.asarray(Wk, np.float32)
    bk = np.asarray(bk, np.float32)
    Wv = np.asarray(Wv, np.float32)
    bv = np.asarray(bv, np.float32)
    Wp = np.asarray(Wp, np.float32)
    bp = np.asarray(bp, np.float32)
    B, T, C = x.shape

    _patch_act_tables()
    nc = bacc.Bacc("TRN2", target_bir_lowering=False, debug=False,
                   num_devices=N_CORES)
    build_attention_kernel(nc, T=T, C=C)
    nc.compile()

    in_maps = make_core_inputs(x, Wq, bq, Wk, bk, Wv, bv, Wp, T=T, C=C)
    res = run_bass_kernel_spmd(nc, in_maps, list(range(N_CORES)))

    out = np.zeros((B, T, C), np.float32)
    for c in range(N_CORES):
        out[c // 4] += res.results[c]["z"]
    out += bp[None, None, :]
    return out


if __name__ == "__main__":
    import reference

    inputs = reference.setup_inputs()
    expected = np.asarray(reference.reference(**inputs))
    actual = kernel(**{k: np.asarray(v) for k, v in inputs.items()})
    err = np.abs(actual - expected).max() / np.abs(expected).max()
    print("Relative error:", err)
